# revision 43
# baseline (speedup 1.0000x reference)
"""AFT (attention-free transformer) block on 8 TRN2 NeuronCores.

Reference computation (T=2048, B=4, D=1024):
    qkv = data @ W_qkv + b_qkv ; q,k,v = split(qkv)
    num = exp(pb - max_pb) @ (exp(k - max_k) * v)    (contraction over key pos j)
    den = exp(pb - max_pb) @ exp(k - max_k)
    out = (sigmoid(q) * num / den) @ W_out + b_out
The max shifts cancel exactly in num/den and the value ranges are tiny
(|k| <~ 4, |pb| <~ 0.12), so the kernel drops them. Compute is bf16 with
fp32 PSUM accumulation (rel err ~4e-3 vs the fp32 reference).

Sharding: hybrid (sequence-half x batch). Core r = 2b + h owns batch b and
query rows i in [h*1024, (h+1)*1024). Each core projects q/k/v for its own
1024 tokens; the 8 cores all-gather exp(k) and exp(k)*v (bf16, two pipelined
j-half chunks); each core then reads back ONLY its batch's slice of the
gathered buffer (8MB instead of 32MB) via indirect DMAs whose row indices
are a per-core host input (gidx) — the SPMD graph stays uniform while the
rank blocks read differ per core. sigmoid(q) needs no data movement at all
because batch is fixed per core.

Pipeline (driven by the ~160us AllGather wire time, the binding constraint):
  - k/v columns are projected first (q deferred) so AG chunk 0 triggers
    ~55us in; the two 2MB collectives then stream back to back.
  - num/den accumulation is split into three passes so the PE never waits
    on the wire: B1 consumes the core's OWN eight j-tiles straight out of
    its local cc_in staging buffers (no collective dependency), B2 adds the
    four gathered chunk-0 other-half tiles once AG0 lands, B3 adds the four
    chunk-1 other-half tiles once AG1 lands. Partials are spilled to SBUF
    as bf16 between passes and merged back into PSUM with identity-matmuls
    (PSUM += I.T @ spill), keeping the DVE epilogue chain short; groups run
    in pairs so one group's matmuls hide the other's epilogue. The host
    permutes each core's pbT slice into slot order so the j-accumulation
    order matches the tile sources uniformly across cores.
  - every matmul reuses one stationary (lhsT) load for 2-4 N=512 moving
    passes (ldw-opt is off in this compile config, so LDWEIGHTS serialize).
  - y is token-major; the [d, i] transposes for the output projection run
    as PE transposes through 2 dedicated PSUM banks.
"""

import numpy as np
import ml_dtypes

from concourse import bacc, bass, mybir, tile
from concourse.bass_utils import run_bass_kernel_spmd
from concourse.masks import make_identity

BF16 = mybir.dt.bfloat16
F32 = mybir.dt.float32
I32 = mybir.dt.int32
AF = mybir.ActivationFunctionType

N_CORES = 8
T, B, D = 2048, 4, 1024
TOK = 1024                 # tokens per core: 1024 query rows of one batch
KT = D // 128              # 8 contraction tiles for d
NG = TOK // 128            # 8 query-tile groups

_cache = {}


def build(with_qkv_bias: bool, with_out_bias: bool):
    nc = bacc.Bacc(None, target_bir_lowering=False)

    dataT_d = nc.dram_tensor("dataT", [D, TOK], BF16, kind="ExternalInput")
    wkv_d = nc.dram_tensor("wkv", [D, 2 * D], BF16, kind="ExternalInput")
    wq_d = nc.dram_tensor("wq", [D, D], BF16, kind="ExternalInput")
    pbT_d = nc.dram_tensor("pbT", [T, TOK], BF16, kind="ExternalInput")
    wout_d = nc.dram_tensor("wout", [D, D], BF16, kind="ExternalInput")
    gidx_d = nc.dram_tensor("gidx", [128, 24], I32, kind="ExternalInput")
    out_d = nc.dram_tensor("out", [TOK, D], F32, kind="ExternalOutput")
    if with_qkv_bias:
        bkv_d = nc.dram_tensor("bkv", [1, 2 * D], BF16, kind="ExternalInput")
        bq_d = nc.dram_tensor("bq", [1, D], BF16, kind="ExternalInput")
    if with_out_bias:
        bout_d = nc.dram_tensor("bout", [1, D], BF16, kind="ExternalInput")

    with tile.TileContext(nc) as tc:
        with (
            tc.tile_pool(name="persist", bufs=1) as pp,
            tc.tile_pool(name="psum", bufs=6, space="PSUM") as psp,
            tc.tile_pool(name="psum_tr", bufs=2, space="PSUM") as pstr,
            tc.tile_pool(name="dram", bufs=1, space="DRAM") as dram,
        ):
            # ---- persistent SBUF tensors ----
            ident = pp.tile([128, 128], BF16, name="ident", tag="ident")
            make_identity(nc, ident[:])
            gidx = pp.tile([128, 24], I32, name="gidx", tag="gidx")
            wout = [pp.tile([128, D], BF16, name=f"wout{k}", tag=f"wout{k}")
                    for k in range(KT)]
            pbe = [pp.tile([128, TOK], BF16, name=f"pbe{t}", tag=f"pbe{t}")
                   for t in range(T // 128)]
            sq_t = [pp.tile([128, D], BF16, name=f"sq{m}", tag=f"sq{m}")
                    for m in range(NG)]
            if with_qkv_bias or with_out_bias:
                ones1 = pp.tile([1, 128], BF16, name="ones1", tag="ones1")
                nc.gpsimd.memset(ones1[:], 1.0)
            if with_qkv_bias:
                bkv = pp.tile([1, 2 * D], BF16, name="bkv", tag="bkv")
                nc.sync.dma_start(bkv[:], bkv_d[:])
                bq = pp.tile([1, D], BF16, name="bq", tag="bq")
                nc.sync.dma_start(bq[:], bq_d[:])
            if with_out_bias:
                bout = pp.tile([1, D], BF16, name="bout", tag="bout")
                nc.sync.dma_start(bout[:], bout_d[:])

            # collective bounce buffers: two token-half chunks of [ek | ekv]
            cc_in = [dram.tile([TOK, D], BF16, name=f"cc_in{x}") for x in range(2)]
            cc_out = [dram.tile([N_CORES * TOK, D], BF16, name=f"cc_out{x}",
                                addr_space="Shared") for x in range(2)]

            # ---- phase A: qkv projection, k/v first ----
            with tc.tile_pool(name="phaseA", bufs=1) as pa:
                dataT = [pa.tile([128, TOK], BF16, name=f"dataT{k}",
                                 tag=f"dataT{k}") for k in range(KT)]
                wkv = [pa.tile([128, 2 * D], BF16, name=f"wkv{k}",
                               tag=f"wkv{k}") for k in range(KT)]
                wq = [pa.tile([128, D], BF16, name=f"wq{k}", tag=f"wq{k}")
                      for k in range(KT)]
                # kv weights first so pass 1 can start after ~6MB of DMA;
                # wkv rides the scalar HWDGE queue so the two load streams
                # drain in parallel
                for k in range(KT):
                    nc.sync.dma_start(dataT[k][:], dataT_d[k * 128:(k + 1) * 128, :])
                    nc.scalar.dma_start(wkv[k][:], wkv_d[k * 128:(k + 1) * 128, :])
                for k in range(KT):
                    nc.sync.dma_start(wq[k][:], wq_d[k * 128:(k + 1) * 128, :])
                # pbT loads land in the startup window, off the AG0 wire window
                praw = [pa.tile([128, TOK], BF16, name=f"praw{t}",
                                tag=f"praw{t}") for t in range(T // 128)]
                for t in range(T // 128):
                    nc.sync.dma_start(praw[t][:], pbT_d[t * 128:(t + 1) * 128, :])

                # pass 1: k and v chunks -> exp(k), exp(k)*v -> cc_in -> AG
                for m in range(NG):  # token tile
                    ek = pa.tile([128, D], BF16, name=f"ek{m}", tag="ek", bufs=3)
                    vv = pa.tile([128, D], BF16, name=f"vv{m}", tag="vv", bufs=3)
                    ekv = pa.tile([128, D], BF16, name=f"ekv{m}", tag="ekv", bufs=3)
                    ps = [psp.tile([128, 512], F32, name=f"ps{m}_{i}",
                                   tag="ps") for i in range(4)]
                    for k in range(KT):
                        for i in range(4):
                            nc.tensor.matmul(
                                ps[i][:], dataT[k][:, m * 128:(m + 1) * 128],
                                wkv[k][:, i * 512:(i + 1) * 512],
                                start=(k == 0),
                                stop=(k == KT - 1 and not with_qkv_bias),
                            )
                    if with_qkv_bias:
                        for i in range(4):
                            nc.tensor.matmul(
                                ps[i][:], ones1[:], bkv[:, i * 512:(i + 1) * 512],
                                start=False, stop=True,
                            )
                    for i in range(2):
                        nc.scalar.activation(
                            ek[:, i * 512:(i + 1) * 512], ps[i][:], AF.Exp)
                        nc.vector.tensor_copy(
                            vv[:, i * 512:(i + 1) * 512], ps[2 + i][:])
                    nc.vector.tensor_mul(ekv[:], ek[:], vv[:])
                    # chunk x = m//4 holds token rows [x*512,(x+1)*512):
                    # layout [ek half | ekv half]
                    x, mm = m // 4, m % 4
                    nc.sync.dma_start(
                        cc_in[x][mm * 128:(mm + 1) * 128, :], ek[:])
                    nc.sync.dma_start(
                        cc_in[x][512 + mm * 128:512 + (mm + 1) * 128, :], ekv[:])
                    if m in (3, 7):
                        nc.gpsimd.collective_compute(
                            "AllGather", mybir.AluOpType.bypass,
                            replica_groups=[list(range(N_CORES))],
                            ins=[cc_in[m // 4][:].opt()],
                            outs=[cc_out[m // 4][:].opt()],
                        )

                # exp(pbT) — ACT is idle here, between the k-exp chains
                for t in range(T // 128):
                    nc.scalar.activation(pbe[t][:], praw[t][:], AF.Exp)

                # pass 2: q chunks + sigmoid — overlaps the collectives;
                # the second half is emitted after the even pass to fill the
                # wait-for-AG1 bubble
                def q_group(m):
                    sq = sq_t[m]
                    ps = [psp.tile([128, 512], F32, name=f"psq{m}_{i}",
                                   tag="ps") for i in range(2)]
                    for k in range(KT):
                        for i in range(2):
                            nc.tensor.matmul(
                                ps[i][:], dataT[k][:, m * 128:(m + 1) * 128],
                                wq[k][:, i * 512:(i + 1) * 512],
                                start=(k == 0),
                                stop=(k == KT - 1 and not with_qkv_bias),
                            )
                    if with_qkv_bias:
                        for i in range(2):
                            nc.tensor.matmul(
                                ps[i][:], ones1[:], bq[:, i * 512:(i + 1) * 512],
                                start=False, stop=True,
                            )
                    for i in range(2):
                        nc.scalar.activation(
                            sq[:, i * 512:(i + 1) * 512], ps[i][:], AF.Sigmoid)

                for m in range(NG):
                    q_group(m)


            # ---- phase B: num/den + y + output projection ----
            # pbe tiles are SLOT-ordered (host-permuted pbT rows):
            #   s in [0,8):   chunk-0 j-tile s (absolute half s//4, tile s%4)
            #   s in [8,12):  chunk-1 OWN-half tile s-8   (local in cc_in[1])
            #   s in [12,16): chunk-1 OTHER-half tile s-12 (needs AG chunk 1)
            # gidx cols: 0-15 chunk-0 (half*8+u), 16-23 chunk-1-other
            # (half*4+o). The own-half chunk-1 tiles are read straight from
            # this core's cc_in[1], before the second collective lands.
            with tc.tile_pool(name="phaseB", bufs=1) as pbp:
                def gload(x, cols, nt, tagp):
                    tiles = []
                    for u in range(nt):
                        g = pbp.tile([128, 2048], BF16, name=f"ekg{tagp}{u}",
                                     tag="ekg", bufs=12)
                        for half in range(2):
                            col = cols + half * nt + u
                            nc.gpsimd.indirect_dma_start(
                                out=g[:, half * D:(half + 1) * D],
                                out_offset=None,
                                in_=cc_out[x][:],
                                in_offset=bass.IndirectOffsetOnAxis(
                                    ap=gidx[:, col:col + 1], axis=0),
                            )
                        tiles.append(g)
                    return tiles

                spill = {}

                def nd_mms(m2, tiles, merge_sp, tagp):
                    pn = [psp.tile([128, 512], F32, name=f"pn{tagp}{m2}{i}",
                                   tag="ps") for i in range(2)]
                    pd = [psp.tile([128, 512], F32, name=f"pd{tagp}{m2}{i}",
                                   tag="ps") for i in range(2)]
                    nt = len(tiles)
                    for u in range(nt):
                        tile_u, slot = tiles[u]
                        pb_t = pbe[slot]
                        for i in range(2):
                            nc.tensor.matmul(
                                pn[i][:], pb_t[:, m2 * 128:(m2 + 1) * 128],
                                tile_u[:, D + i * 512:D + (i + 1) * 512],
                                start=(u == 0),
                                stop=(u == nt - 1 and merge_sp is None))
                            nc.tensor.matmul(
                                pd[i][:], pb_t[:, m2 * 128:(m2 + 1) * 128],
                                tile_u[:, i * 512:(i + 1) * 512],
                                start=(u == 0),
                                stop=(u == nt - 1 and merge_sp is None))
                    if merge_sp is not None:
                        for i in range(2):
                            nc.tensor.matmul(
                                pn[i][:], ident[:],
                                merge_sp[:, i * 512:(i + 1) * 512],
                                start=False, stop=True)
                            nc.tensor.matmul(
                                pd[i][:], ident[:],
                                merge_sp[:, D + i * 512:D + (i + 1) * 512],
                                start=False, stop=True)
                    return pn, pd

                def to_spill(m2, pn, pd):
                    if m2 in spill:
                        sp = spill[m2]
                    else:
                        sp = pbp.tile([128, 4 * 512], BF16, name=f"sp{m2}",
                                      tag=f"sp{m2}")
                        spill[m2] = sp
                    for i in range(2):
                        nc.scalar.copy(sp[:, i * 512:(i + 1) * 512], pn[i][:])
                        nc.scalar.copy(
                            sp[:, D + i * 512:D + (i + 1) * 512], pd[i][:])

                # pass B1: BOTH chunks' own-half tiles, straight from this
                # core's cc_in — runs while the collectives are on the wire
                own = []
                for x in range(2):
                    for o in range(4):
                        g = pbp.tile([128, 2048], BF16, name=f"ekgo{x}{o}",
                                     tag="ekg", bufs=12)
                        nc.sync.dma_start(
                            g[:, :D], cc_in[x][o * 128:(o + 1) * 128, :])
                        nc.sync.dma_start(
                            g[:, D:],
                            cc_in[x][512 + o * 128:512 + (o + 1) * 128, :])
                        own.append((g, 8 * x + o))
                # gidx/wout: needed later than the B1 tiles above
                nc.sync.dma_start(gidx[:], gidx_d[:])
                for k in range(KT):
                    nc.sync.dma_start(wout[k][:], wout_d[k * 128:(k + 1) * 128, :])
                for pair in range(NG // 2):
                    g0, g1 = 2 * pair, 2 * pair + 1
                    a0 = nd_mms(g0, own, None, "A")
                    to_spill(g0, *a0)
                    a1 = nd_mms(g1, own, None, "A")
                    to_spill(g1, *a1)

                # pass B2: chunk-0 other-half tiles (after AG chunk 0),
                # merging the B1 partials back in on the PE
                ekg0 = [(g, 4 + o) for o, g in enumerate(gload(0, 0, 4, "z"))]
                for pair in range(NG // 2):
                    g0, g1 = 2 * pair, 2 * pair + 1
                    a0 = nd_mms(g0, ekg0, spill[g0], "B")
                    to_spill(g0, *a0)
                    a1 = nd_mms(g1, ekg0, spill[g1], "B")
                    to_spill(g1, *a1)

                # pass B3: chunk-1 other-half tiles (after AG chunk 1),
                # epilogue, transpose, output projection
                ekg1 = [(g, 12 + o) for o, g in enumerate(gload(1, 8, 4, "o"))]

                def tail(m2, yv):
                    yT = [pbp.tile([128, 128], BF16, name=f"yT{m2}_{k}",
                                   tag=f"yT{k}", bufs=2) for k in range(KT)]
                    for k in range(KT):
                        pt = pstr.tile([128, 128], BF16, name=f"pt{m2}{k}",
                                       tag="tr")
                        nc.tensor.transpose(
                            pt[:], yv[:, k * 128:(k + 1) * 128], ident[:])
                        nc.vector.tensor_copy(yT[k][:], pt[:])
                    po = [psp.tile([128, 512], F32, name=f"po{m2}_{n}", tag="ps")
                          for n in range(2)]
                    for k in range(KT):
                        for n in range(2):
                            nc.tensor.matmul(
                                po[n][:], yT[k][:],
                                wout[k][:, n * 512:(n + 1) * 512],
                                start=(k == 0),
                                stop=(k == KT - 1 and not with_out_bias))
                    if with_out_bias:
                        for n in range(2):
                            nc.tensor.matmul(
                                po[n][:], ones1[:], bout[:, n * 512:(n + 1) * 512],
                                start=False, stop=True)
                    for n in range(2):
                        osb = pbp.tile([128, 512], F32, name=f"osb{m2}_{n}",
                                       tag="osb", bufs=4)
                        nc.scalar.copy(osb[:], po[n][:])
                        nc.sync.dma_start(
                            out_d[m2 * 128:(m2 + 1) * 128,
                                  n * 512:(n + 1) * 512], osb[:])

                def odd_group(m2):
                    pn, pd = nd_mms(m2, ekg1, spill[m2], "C")
                    y = pbp.tile([128, D], BF16, name=f"y{m2}", tag="y", bufs=3)
                    for i in range(2):
                        tn = pbp.tile([128, 512], F32, name=f"tn{m2}{i}",
                                      tag="tn", bufs=3)
                        rec = pbp.tile([128, 512], F32, name=f"rc{m2}{i}",
                                       tag="rc", bufs=3)
                        nc.vector.reciprocal_approx_fast(rec[:], pd[i][:])
                        nc.vector.tensor_mul(tn[:], pn[i][:], rec[:])
                        nc.vector.tensor_mul(
                            y[:, i * 512:(i + 1) * 512], tn[:],
                            sq_t[m2][:, i * 512:(i + 1) * 512])
                    return y

                for pair in range(NG // 2):
                    g0, g1 = 2 * pair, 2 * pair + 1
                    y0 = odd_group(g0)
                    y1 = odd_group(g1)
                    tail(g0, y0)
                    tail(g1, y1)

    nc.compile()
    return nc


def _prep_inputs(data, W_qkv, b_qkv, pos_bias_param, W_out, b_out):
    bf = ml_dtypes.bfloat16
    data = np.asarray(data, np.float32)
    W_qkv = np.asarray(W_qkv, np.float32)
    b_qkv = np.asarray(b_qkv, np.float32)
    pos_bias_param = np.asarray(pos_bias_param, np.float32)
    W_out = np.asarray(W_out, np.float32)
    b_out = np.asarray(b_out, np.float32)

    with_qkv_bias = bool(np.any(b_qkv))
    with_out_bias = bool(np.any(b_out))

    wq = np.ascontiguousarray(W_qkv[:, :D]).astype(bf)
    wkv = np.ascontiguousarray(W_qkv[:, D:]).astype(bf)
    wout = W_out.astype(bf)
    pbT = np.ascontiguousarray(pos_bias_param.T)  # [j, i]

    p = np.arange(128)
    in_maps = []
    for r in range(N_CORES):
        b, h = r // 2, r % 2
        isl = slice(h * TOK, (h + 1) * TOK)
        dT = np.ascontiguousarray(
            data[isl, b, :].T).astype(bf)                    # [d_in, tok]
        pbT_c = np.ascontiguousarray(pbT[:, isl]).astype(bf)  # [j, i_loc]
        # slot-permute pbT rows: per chunk x, slots 0-3 = own half,
        # 4-7 = other half; pbe slot s = 8*x + within-chunk slot
        rows = []
        for s in range(16):
            x, ss = s // 8, s % 8
            hp = h if ss < 4 else 1 - h
            j0 = hp * 1024 + 512 * x + (ss % 4) * 128
            rows.append(pbT_c[j0:j0 + 128])
        pbT_c = np.ascontiguousarray(np.concatenate(rows, axis=0))
        # gidx: cols x*8 + half*4 + o -> chunk-x other-half tile o
        gidx = np.zeros((128, 24), np.int32)
        for x in range(2):
            for half in range(2):
                for o in range(4):
                    base = (2 * b + (1 - h)) * 1024 + half * 512 + o * 128
                    gidx[:, x * 8 + half * 4 + o] = base + p
        m = {"dataT": dT, "wq": wq, "wkv": wkv, "pbT": pbT_c, "wout": wout,
             "gidx": gidx}
        if with_qkv_bias:
            m["bq"] = np.ascontiguousarray(b_qkv[:D]).reshape(1, D).astype(bf)
            m["bkv"] = np.ascontiguousarray(b_qkv[D:]).reshape(1, 2 * D).astype(bf)
        if with_out_bias:
            m["bout"] = b_out.reshape(1, D).astype(bf)
        in_maps.append(m)
    return in_maps, with_qkv_bias, with_out_bias


def run(data, W_qkv, b_qkv, pos_bias_param, W_out, b_out, **spmd_kwargs):
    in_maps, wb, ob = _prep_inputs(data, W_qkv, b_qkv, pos_bias_param, W_out, b_out)
    key = (wb, ob)
    if key not in _cache:
        _cache[key] = build(wb, ob)
    nc = _cache[key]
    res = run_bass_kernel_spmd(nc, in_maps, core_ids=list(range(N_CORES)),
                               **spmd_kwargs)
    out = np.empty((T, B, D), np.float32)
    for r in range(N_CORES):
        b, h = r // 2, r % 2
        out[h * TOK:(h + 1) * TOK, b, :] = res.results[r]["out"]
    return out, res


def kernel(data, W_qkv, b_qkv, pos_bias_param, W_out, b_out):
    out, _ = run(data, W_qkv, b_qkv, pos_bias_param, W_out, b_out)
    return out


# revision 44
# speedup vs baseline: 1.0268x; 1.0268x over previous
"""AFT (attention-free transformer) block on 8 TRN2 NeuronCores.

Reference computation (T=2048, B=4, D=1024):
    qkv = data @ W_qkv + b_qkv ; q,k,v = split(qkv)
    num = exp(pb - max_pb) @ (exp(k - max_k) * v)    (contraction over key pos j)
    den = exp(pb - max_pb) @ exp(k - max_k)
    out = (sigmoid(q) * num / den) @ W_out + b_out
The max shifts cancel exactly in num/den and the value ranges are tiny
(|k| <~ 4, |pb| <~ 0.12), so the kernel drops them. Compute is bf16 with
fp32 PSUM accumulation (rel err ~4e-3 vs the fp32 reference).

Sharding: hybrid (sequence-half x batch). Core r = 2b + h owns batch b and
query rows i in [h*1024, (h+1)*1024). Each core projects q/k/v for its own
1024 tokens; the 8 cores all-gather exp(k) and exp(k)*v (bf16, two pipelined
j-half chunks); each core then reads back ONLY its batch's slice of the
gathered buffer (8MB instead of 32MB) via indirect DMAs whose row indices
are a per-core host input (gidx) — the SPMD graph stays uniform while the
rank blocks read differ per core. sigmoid(q) needs no data movement at all
because batch is fixed per core.

Pipeline (driven by the ~160us AllGather wire time, the binding constraint):
  - k/v columns are projected first (q deferred) so AG chunk 0 triggers
    ~55us in; the two 2MB collectives then stream back to back.
  - num/den accumulation is split into three passes so the PE never waits
    on the wire: B1 consumes the core's OWN eight j-tiles straight out of
    its local cc_in staging buffers (no collective dependency), B2 adds the
    four gathered chunk-0 other-half tiles once AG0 lands, B3 adds the four
    chunk-1 other-half tiles once AG1 lands. Partials are spilled to SBUF
    as bf16 between passes and merged back into PSUM with identity-matmuls
    (PSUM += I.T @ spill), keeping the DVE epilogue chain short; groups run
    in pairs so one group's matmuls hide the other's epilogue. The host
    permutes each core's pbT slice into slot order so the j-accumulation
    order matches the tile sources uniformly across cores.
  - every matmul reuses one stationary (lhsT) load for 2-4 N=512 moving
    passes (ldw-opt is off in this compile config, so LDWEIGHTS serialize).
  - y is token-major; the [d, i] transposes for the output projection run
    as PE transposes through 2 dedicated PSUM banks.
"""

import numpy as np
import ml_dtypes

from concourse import bacc, bass, mybir, tile
from concourse.bass_utils import run_bass_kernel_spmd
from concourse.masks import make_identity

BF16 = mybir.dt.bfloat16
F32 = mybir.dt.float32
I32 = mybir.dt.int32
AF = mybir.ActivationFunctionType

N_CORES = 8
T, B, D = 2048, 4, 1024
TOK = 1024                 # tokens per core: 1024 query rows of one batch
KT = D // 128              # 8 contraction tiles for d
NG = TOK // 128            # 8 query-tile groups

_cache = {}


def build(with_qkv_bias: bool, with_out_bias: bool):
    nc = bacc.Bacc(None, target_bir_lowering=False)

    dataT_d = nc.dram_tensor("dataT", [D, TOK], BF16, kind="ExternalInput")
    wkv_d = nc.dram_tensor("wkv", [D, 2 * D], BF16, kind="ExternalInput")
    wq_d = nc.dram_tensor("wq", [D, D], BF16, kind="ExternalInput")
    pbT_d = nc.dram_tensor("pbT", [T, TOK], BF16, kind="ExternalInput")
    wout_d = nc.dram_tensor("wout", [D, D], BF16, kind="ExternalInput")
    gidx_d = nc.dram_tensor("gidx", [128, 24], I32, kind="ExternalInput")
    out_d = nc.dram_tensor("out", [TOK, D], F32, kind="ExternalOutput")
    if with_qkv_bias:
        bkv_d = nc.dram_tensor("bkv", [1, 2 * D], BF16, kind="ExternalInput")
        bq_d = nc.dram_tensor("bq", [1, D], BF16, kind="ExternalInput")
    if with_out_bias:
        bout_d = nc.dram_tensor("bout", [1, D], BF16, kind="ExternalInput")

    with tile.TileContext(nc) as tc:
        with (
            tc.tile_pool(name="persist", bufs=1) as pp,
            tc.tile_pool(name="psum", bufs=6, space="PSUM") as psp,
            tc.tile_pool(name="psum_tr", bufs=2, space="PSUM") as pstr,
            tc.tile_pool(name="dram", bufs=1, space="DRAM") as dram,
        ):
            # ---- persistent SBUF tensors ----
            ident = pp.tile([128, 128], BF16, name="ident", tag="ident")
            make_identity(nc, ident[:])
            gidx = pp.tile([128, 24], I32, name="gidx", tag="gidx")
            wout = [pp.tile([128, D], BF16, name=f"wout{k}", tag=f"wout{k}")
                    for k in range(KT)]
            pbe = [pp.tile([128, TOK], BF16, name=f"pbe{t}", tag=f"pbe{t}")
                   for t in range(T // 128)]
            sq_t = [pp.tile([128, D], BF16, name=f"sq{m}", tag=f"sq{m}")
                    for m in range(NG)]
            if with_qkv_bias or with_out_bias:
                ones1 = pp.tile([1, 128], BF16, name="ones1", tag="ones1")
                nc.gpsimd.memset(ones1[:], 1.0)
            if with_qkv_bias:
                bkv = pp.tile([1, 2 * D], BF16, name="bkv", tag="bkv")
                nc.sync.dma_start(bkv[:], bkv_d[:])
                bq = pp.tile([1, D], BF16, name="bq", tag="bq")
                nc.sync.dma_start(bq[:], bq_d[:])
            if with_out_bias:
                bout = pp.tile([1, D], BF16, name="bout", tag="bout")
                nc.sync.dma_start(bout[:], bout_d[:])

            # collective bounce buffers: two token-half chunks of [ek | ekv]
            cc_in = [dram.tile([TOK, D], BF16, name=f"cc_in{x}") for x in range(2)]
            cc_out = [dram.tile([N_CORES * TOK, D], BF16, name=f"cc_out{x}",
                                addr_space="Shared") for x in range(2)]

            # ---- phase A: qkv projection, k/v first ----
            with tc.tile_pool(name="phaseA", bufs=1) as pa:
                dataT = [pa.tile([128, TOK], BF16, name=f"dataT{k}",
                                 tag=f"dataT{k}") for k in range(KT)]
                wkv = [pa.tile([128, 2 * D], BF16, name=f"wkv{k}",
                               tag=f"wkv{k}") for k in range(KT)]
                wq = [pa.tile([128, D], BF16, name=f"wq{k}", tag=f"wq{k}")
                      for k in range(KT)]
                # kv weights first so pass 1 can start after ~6MB of DMA;
                # wkv rides the scalar HWDGE queue so the two load streams
                # drain in parallel
                for k in range(KT):
                    nc.sync.dma_start(dataT[k][:], dataT_d[k * 128:(k + 1) * 128, :])
                    nc.scalar.dma_start(wkv[k][:], wkv_d[k * 128:(k + 1) * 128, :])
                for k in range(KT):
                    nc.sync.dma_start(wq[k][:], wq_d[k * 128:(k + 1) * 128, :])
                # pbT loads land in the startup window, off the AG0 wire window
                praw = [pa.tile([128, TOK], BF16, name=f"praw{t}",
                                tag=f"praw{t}") for t in range(T // 128)]
                for t in range(T // 128):
                    nc.scalar.dma_start(praw[t][:], pbT_d[t * 128:(t + 1) * 128, :])

                # pass 1: k and v chunks -> exp(k), exp(k)*v -> cc_in -> AG
                for m in range(NG):  # token tile
                    ek = pa.tile([128, D], BF16, name=f"ek{m}", tag="ek", bufs=3)
                    vv = pa.tile([128, D], BF16, name=f"vv{m}", tag="vv", bufs=3)
                    ekv = pa.tile([128, D], BF16, name=f"ekv{m}", tag="ekv", bufs=3)
                    ps = [psp.tile([128, 512], F32, name=f"ps{m}_{i}",
                                   tag="ps") for i in range(4)]
                    for k in range(KT):
                        for i in range(4):
                            nc.tensor.matmul(
                                ps[i][:], dataT[k][:, m * 128:(m + 1) * 128],
                                wkv[k][:, i * 512:(i + 1) * 512],
                                start=(k == 0),
                                stop=(k == KT - 1 and not with_qkv_bias),
                            )
                    if with_qkv_bias:
                        for i in range(4):
                            nc.tensor.matmul(
                                ps[i][:], ones1[:], bkv[:, i * 512:(i + 1) * 512],
                                start=False, stop=True,
                            )
                    for i in range(2):
                        nc.scalar.activation(
                            ek[:, i * 512:(i + 1) * 512], ps[i][:], AF.Exp)
                        nc.vector.tensor_copy(
                            vv[:, i * 512:(i + 1) * 512], ps[2 + i][:])
                    nc.vector.tensor_mul(ekv[:], ek[:], vv[:])
                    # chunk x = m//4 holds token rows [x*512,(x+1)*512):
                    # layout [ek half | ekv half]
                    x, mm = m // 4, m % 4
                    nc.sync.dma_start(
                        cc_in[x][mm * 128:(mm + 1) * 128, :], ek[:])
                    nc.sync.dma_start(
                        cc_in[x][512 + mm * 128:512 + (mm + 1) * 128, :], ekv[:])
                    if m in (3, 7):
                        nc.gpsimd.collective_compute(
                            "AllGather", mybir.AluOpType.bypass,
                            replica_groups=[list(range(N_CORES))],
                            ins=[cc_in[m // 4][:].opt()],
                            outs=[cc_out[m // 4][:].opt()],
                        )

                # exp(pbT) — ACT is idle here, between the k-exp chains
                for t in range(T // 128):
                    nc.scalar.activation(pbe[t][:], praw[t][:], AF.Exp)

                # pass 2: q chunks + sigmoid — overlaps the collectives;
                # the second half is emitted after the even pass to fill the
                # wait-for-AG1 bubble
                def q_group(m):
                    sq = sq_t[m]
                    ps = [psp.tile([128, 512], F32, name=f"psq{m}_{i}",
                                   tag="ps") for i in range(2)]
                    for k in range(KT):
                        for i in range(2):
                            nc.tensor.matmul(
                                ps[i][:], dataT[k][:, m * 128:(m + 1) * 128],
                                wq[k][:, i * 512:(i + 1) * 512],
                                start=(k == 0),
                                stop=(k == KT - 1 and not with_qkv_bias),
                            )
                    if with_qkv_bias:
                        for i in range(2):
                            nc.tensor.matmul(
                                ps[i][:], ones1[:], bq[:, i * 512:(i + 1) * 512],
                                start=False, stop=True,
                            )
                    for i in range(2):
                        nc.scalar.activation(
                            sq[:, i * 512:(i + 1) * 512], ps[i][:], AF.Sigmoid)

                for m in range(NG):
                    q_group(m)


            # ---- phase B: num/den + y + output projection ----
            # pbe tiles are SLOT-ordered (host-permuted pbT rows):
            #   s in [0,8):   chunk-0 j-tile s (absolute half s//4, tile s%4)
            #   s in [8,12):  chunk-1 OWN-half tile s-8   (local in cc_in[1])
            #   s in [12,16): chunk-1 OTHER-half tile s-12 (needs AG chunk 1)
            # gidx cols: 0-15 chunk-0 (half*8+u), 16-23 chunk-1-other
            # (half*4+o). The own-half chunk-1 tiles are read straight from
            # this core's cc_in[1], before the second collective lands.
            with tc.tile_pool(name="phaseB", bufs=1) as pbp:
                def gload(x, cols, nt, tagp):
                    tiles = []
                    for u in range(nt):
                        g = pbp.tile([128, 2048], BF16, name=f"ekg{tagp}{u}",
                                     tag="ekg", bufs=12)
                        for half in range(2):
                            col = cols + half * nt + u
                            nc.gpsimd.indirect_dma_start(
                                out=g[:, half * D:(half + 1) * D],
                                out_offset=None,
                                in_=cc_out[x][:],
                                in_offset=bass.IndirectOffsetOnAxis(
                                    ap=gidx[:, col:col + 1], axis=0),
                            )
                        tiles.append(g)
                    return tiles

                spill = {}

                def nd_mms(m2, tiles, merge_sp, tagp):
                    pn = [psp.tile([128, 512], F32, name=f"pn{tagp}{m2}{i}",
                                   tag="ps") for i in range(2)]
                    pd = [psp.tile([128, 512], F32, name=f"pd{tagp}{m2}{i}",
                                   tag="ps") for i in range(2)]
                    nt = len(tiles)
                    for u in range(nt):
                        tile_u, slot = tiles[u]
                        pb_t = pbe[slot]
                        for i in range(2):
                            nc.tensor.matmul(
                                pn[i][:], pb_t[:, m2 * 128:(m2 + 1) * 128],
                                tile_u[:, D + i * 512:D + (i + 1) * 512],
                                start=(u == 0),
                                stop=(u == nt - 1 and merge_sp is None))
                            nc.tensor.matmul(
                                pd[i][:], pb_t[:, m2 * 128:(m2 + 1) * 128],
                                tile_u[:, i * 512:(i + 1) * 512],
                                start=(u == 0),
                                stop=(u == nt - 1 and merge_sp is None))
                    if merge_sp is not None:
                        for i in range(2):
                            nc.tensor.matmul(
                                pn[i][:], ident[:],
                                merge_sp[:, i * 512:(i + 1) * 512],
                                start=False, stop=True)
                            nc.tensor.matmul(
                                pd[i][:], ident[:],
                                merge_sp[:, D + i * 512:D + (i + 1) * 512],
                                start=False, stop=True)
                    return pn, pd

                def to_spill(m2, pn, pd):
                    if m2 in spill:
                        sp = spill[m2]
                    else:
                        sp = pbp.tile([128, 4 * 512], BF16, name=f"sp{m2}",
                                      tag=f"sp{m2}")
                        spill[m2] = sp
                    for i in range(2):
                        nc.scalar.copy(sp[:, i * 512:(i + 1) * 512], pn[i][:])
                        nc.scalar.copy(
                            sp[:, D + i * 512:D + (i + 1) * 512], pd[i][:])

                # pass B1: BOTH chunks' own-half tiles, straight from this
                # core's cc_in — runs while the collectives are on the wire
                own = []
                for x in range(2):
                    for o in range(4):
                        g = pbp.tile([128, 2048], BF16, name=f"ekgo{x}{o}",
                                     tag="ekg", bufs=12)
                        nc.sync.dma_start(
                            g[:, :D], cc_in[x][o * 128:(o + 1) * 128, :])
                        nc.sync.dma_start(
                            g[:, D:],
                            cc_in[x][512 + o * 128:512 + (o + 1) * 128, :])
                        own.append((g, 8 * x + o))
                # gidx/wout: needed later than the B1 tiles above
                nc.sync.dma_start(gidx[:], gidx_d[:])
                for k in range(KT):
                    nc.sync.dma_start(wout[k][:], wout_d[k * 128:(k + 1) * 128, :])
                for pair in range(NG // 2):
                    g0, g1 = 2 * pair, 2 * pair + 1
                    a0 = nd_mms(g0, own, None, "A")
                    to_spill(g0, *a0)
                    a1 = nd_mms(g1, own, None, "A")
                    to_spill(g1, *a1)

                # pass B2: chunk-0 other-half tiles (after AG chunk 0),
                # merging the B1 partials back in on the PE
                ekg0 = [(g, 4 + o) for o, g in enumerate(gload(0, 0, 4, "z"))]
                for pair in range(NG // 2):
                    g0, g1 = 2 * pair, 2 * pair + 1
                    a0 = nd_mms(g0, ekg0, spill[g0], "B")
                    to_spill(g0, *a0)
                    a1 = nd_mms(g1, ekg0, spill[g1], "B")
                    to_spill(g1, *a1)

                # pass B3: chunk-1 other-half tiles (after AG chunk 1),
                # epilogue, transpose, output projection
                ekg1 = [(g, 12 + o) for o, g in enumerate(gload(1, 8, 4, "o"))]

                def tail(m2, yv):
                    yT = [pbp.tile([128, 128], BF16, name=f"yT{m2}_{k}",
                                   tag=f"yT{k}", bufs=2) for k in range(KT)]
                    for k in range(KT):
                        pt = pstr.tile([128, 128], BF16, name=f"pt{m2}{k}",
                                       tag="tr")
                        nc.tensor.transpose(
                            pt[:], yv[:, k * 128:(k + 1) * 128], ident[:])
                        nc.vector.tensor_copy(yT[k][:], pt[:])
                    po = [psp.tile([128, 512], F32, name=f"po{m2}_{n}", tag="ps")
                          for n in range(2)]
                    for k in range(KT):
                        for n in range(2):
                            nc.tensor.matmul(
                                po[n][:], yT[k][:],
                                wout[k][:, n * 512:(n + 1) * 512],
                                start=(k == 0),
                                stop=(k == KT - 1 and not with_out_bias))
                    if with_out_bias:
                        for n in range(2):
                            nc.tensor.matmul(
                                po[n][:], ones1[:], bout[:, n * 512:(n + 1) * 512],
                                start=False, stop=True)
                    for n in range(2):
                        osb = pbp.tile([128, 512], F32, name=f"osb{m2}_{n}",
                                       tag="osb", bufs=4)
                        nc.scalar.copy(osb[:], po[n][:])
                        nc.sync.dma_start(
                            out_d[m2 * 128:(m2 + 1) * 128,
                                  n * 512:(n + 1) * 512], osb[:])

                def odd_group(m2):
                    pn, pd = nd_mms(m2, ekg1, spill[m2], "C")
                    y = pbp.tile([128, D], BF16, name=f"y{m2}", tag="y", bufs=3)
                    for i in range(2):
                        tn = pbp.tile([128, 512], F32, name=f"tn{m2}{i}",
                                      tag="tn", bufs=3)
                        rec = pbp.tile([128, 512], F32, name=f"rc{m2}{i}",
                                       tag="rc", bufs=3)
                        nc.vector.reciprocal_approx_fast(rec[:], pd[i][:])
                        nc.vector.tensor_mul(tn[:], pn[i][:], rec[:])
                        nc.vector.tensor_mul(
                            y[:, i * 512:(i + 1) * 512], tn[:],
                            sq_t[m2][:, i * 512:(i + 1) * 512])
                    return y

                for pair in range(NG // 2):
                    g0, g1 = 2 * pair, 2 * pair + 1
                    y0 = odd_group(g0)
                    y1 = odd_group(g1)
                    tail(g0, y0)
                    tail(g1, y1)

    nc.compile()
    return nc


def _prep_inputs(data, W_qkv, b_qkv, pos_bias_param, W_out, b_out):
    bf = ml_dtypes.bfloat16
    data = np.asarray(data, np.float32)
    W_qkv = np.asarray(W_qkv, np.float32)
    b_qkv = np.asarray(b_qkv, np.float32)
    pos_bias_param = np.asarray(pos_bias_param, np.float32)
    W_out = np.asarray(W_out, np.float32)
    b_out = np.asarray(b_out, np.float32)

    with_qkv_bias = bool(np.any(b_qkv))
    with_out_bias = bool(np.any(b_out))

    wq = np.ascontiguousarray(W_qkv[:, :D]).astype(bf)
    wkv = np.ascontiguousarray(W_qkv[:, D:]).astype(bf)
    wout = W_out.astype(bf)
    pbT = np.ascontiguousarray(pos_bias_param.T)  # [j, i]

    p = np.arange(128)
    in_maps = []
    for r in range(N_CORES):
        b, h = r // 2, r % 2
        isl = slice(h * TOK, (h + 1) * TOK)
        dT = np.ascontiguousarray(
            data[isl, b, :].T).astype(bf)                    # [d_in, tok]
        pbT_c = np.ascontiguousarray(pbT[:, isl]).astype(bf)  # [j, i_loc]
        # slot-permute pbT rows: per chunk x, slots 0-3 = own half,
        # 4-7 = other half; pbe slot s = 8*x + within-chunk slot
        rows = []
        for s in range(16):
            x, ss = s // 8, s % 8
            hp = h if ss < 4 else 1 - h
            j0 = hp * 1024 + 512 * x + (ss % 4) * 128
            rows.append(pbT_c[j0:j0 + 128])
        pbT_c = np.ascontiguousarray(np.concatenate(rows, axis=0))
        # gidx: cols x*8 + half*4 + o -> chunk-x other-half tile o
        gidx = np.zeros((128, 24), np.int32)
        for x in range(2):
            for half in range(2):
                for o in range(4):
                    base = (2 * b + (1 - h)) * 1024 + half * 512 + o * 128
                    gidx[:, x * 8 + half * 4 + o] = base + p
        m = {"dataT": dT, "wq": wq, "wkv": wkv, "pbT": pbT_c, "wout": wout,
             "gidx": gidx}
        if with_qkv_bias:
            m["bq"] = np.ascontiguousarray(b_qkv[:D]).reshape(1, D).astype(bf)
            m["bkv"] = np.ascontiguousarray(b_qkv[D:]).reshape(1, 2 * D).astype(bf)
        if with_out_bias:
            m["bout"] = b_out.reshape(1, D).astype(bf)
        in_maps.append(m)
    return in_maps, with_qkv_bias, with_out_bias


def run(data, W_qkv, b_qkv, pos_bias_param, W_out, b_out, **spmd_kwargs):
    in_maps, wb, ob = _prep_inputs(data, W_qkv, b_qkv, pos_bias_param, W_out, b_out)
    key = (wb, ob)
    if key not in _cache:
        _cache[key] = build(wb, ob)
    nc = _cache[key]
    res = run_bass_kernel_spmd(nc, in_maps, core_ids=list(range(N_CORES)),
                               **spmd_kwargs)
    out = np.empty((T, B, D), np.float32)
    for r in range(N_CORES):
        b, h = r // 2, r % 2
        out[h * TOK:(h + 1) * TOK, b, :] = res.results[r]["out"]
    return out, res


def kernel(data, W_qkv, b_qkv, pos_bias_param, W_out, b_out):
    out, _ = run(data, W_qkv, b_qkv, pos_bias_param, W_out, b_out)
    return out


# revision 45
# speedup vs baseline: 1.0590x; 1.0313x over previous
"""AFT (attention-free transformer) block on 8 TRN2 NeuronCores.

Reference computation (T=2048, B=4, D=1024):
    qkv = data @ W_qkv + b_qkv ; q,k,v = split(qkv)
    num = exp(pb - max_pb) @ (exp(k - max_k) * v)    (contraction over key pos j)
    den = exp(pb - max_pb) @ exp(k - max_k)
    out = (sigmoid(q) * num / den) @ W_out + b_out
The max shifts cancel exactly in num/den and the value ranges are tiny
(|k| <~ 4, |pb| <~ 0.12), so the kernel drops them. Compute is bf16 with
fp32 PSUM accumulation (rel err ~4e-3 vs the fp32 reference).

Sharding: hybrid (sequence-half x batch). Core r = 2b + h owns batch b and
query rows i in [h*1024, (h+1)*1024). Each core projects q/k/v for its own
1024 tokens; the 8 cores all-gather exp(k) and exp(k)*v (bf16, two pipelined
j-half chunks); each core then reads back ONLY its batch's slice of the
gathered buffer (8MB instead of 32MB) via indirect DMAs whose row indices
are a per-core host input (gidx) — the SPMD graph stays uniform while the
rank blocks read differ per core. sigmoid(q) needs no data movement at all
because batch is fixed per core.

Pipeline (driven by the ~160us AllGather wire time, the binding constraint):
  - k/v columns are projected first (q deferred) so AG chunk 0 triggers
    ~55us in; the two 2MB collectives then stream back to back.
  - num/den accumulation is split into three passes so the PE never waits
    on the wire: B1 consumes the core's OWN eight j-tiles straight out of
    its local cc_in staging buffers (no collective dependency), B2 adds the
    four gathered chunk-0 other-half tiles once AG0 lands, B3 adds the four
    chunk-1 other-half tiles once AG1 lands. Partials are spilled to SBUF
    as bf16 between passes and merged back into PSUM with identity-matmuls
    (PSUM += I.T @ spill), keeping the DVE epilogue chain short; groups run
    in pairs so one group's matmuls hide the other's epilogue. The host
    permutes each core's pbT slice into slot order so the j-accumulation
    order matches the tile sources uniformly across cores.
  - every matmul reuses one stationary (lhsT) load for 2-4 N=512 moving
    passes (ldw-opt is off in this compile config, so LDWEIGHTS serialize).
  - y is token-major; the [d, i] transposes for the output projection run
    as PE transposes through 2 dedicated PSUM banks.
"""

import numpy as np
import ml_dtypes

from concourse import bacc, bass, mybir, tile
from concourse.bass_utils import run_bass_kernel_spmd
from concourse.masks import make_identity

BF16 = mybir.dt.bfloat16
F32 = mybir.dt.float32
I32 = mybir.dt.int32
AF = mybir.ActivationFunctionType

N_CORES = 8
T, B, D = 2048, 4, 1024
TOK = 1024                 # tokens per core: 1024 query rows of one batch
KT = D // 128              # 8 contraction tiles for d
NG = TOK // 128            # 8 query-tile groups

_cache = {}


def build(with_qkv_bias: bool, with_out_bias: bool):
    nc = bacc.Bacc(None, target_bir_lowering=False)

    dataT_d = nc.dram_tensor("dataT", [D, TOK], BF16, kind="ExternalInput")
    wkv_d = nc.dram_tensor("wkv", [D, 2 * D], BF16, kind="ExternalInput")
    wq_d = nc.dram_tensor("wq", [D, D], BF16, kind="ExternalInput")
    pbT_d = nc.dram_tensor("pbT", [T, TOK], BF16, kind="ExternalInput")
    wout_d = nc.dram_tensor("wout", [D, D], BF16, kind="ExternalInput")
    gidx_d = nc.dram_tensor("gidx", [128, 24], I32, kind="ExternalInput")
    out_d = nc.dram_tensor("out", [TOK, D], F32, kind="ExternalOutput")
    if with_qkv_bias:
        bkv_d = nc.dram_tensor("bkv", [1, 2 * D], BF16, kind="ExternalInput")
        bq_d = nc.dram_tensor("bq", [1, D], BF16, kind="ExternalInput")
    if with_out_bias:
        bout_d = nc.dram_tensor("bout", [1, D], BF16, kind="ExternalInput")

    with tile.TileContext(nc) as tc:
        with (
            tc.tile_pool(name="persist", bufs=1) as pp,
            tc.tile_pool(name="psum", bufs=6, space="PSUM") as psp,
            tc.tile_pool(name="psum_tr", bufs=2, space="PSUM") as pstr,
            tc.tile_pool(name="dram", bufs=1, space="DRAM") as dram,
        ):
            # ---- persistent SBUF tensors ----
            ident = pp.tile([128, 128], BF16, name="ident", tag="ident")
            make_identity(nc, ident[:])
            gidx = pp.tile([128, 24], I32, name="gidx", tag="gidx")
            wout = [pp.tile([128, D], BF16, name=f"wout{k}", tag=f"wout{k}")
                    for k in range(KT)]
            pbe = [pp.tile([128, TOK], BF16, name=f"pbe{t}", tag=f"pbe{t}")
                   for t in range(T // 128)]
            sq_t = [pp.tile([128, D], BF16, name=f"sq{m}", tag=f"sq{m}")
                    for m in range(NG)]
            if with_qkv_bias or with_out_bias:
                ones1 = pp.tile([1, 128], BF16, name="ones1", tag="ones1")
                nc.gpsimd.memset(ones1[:], 1.0)
            if with_qkv_bias:
                bkv = pp.tile([1, 2 * D], BF16, name="bkv", tag="bkv")
                nc.sync.dma_start(bkv[:], bkv_d[:])
                bq = pp.tile([1, D], BF16, name="bq", tag="bq")
                nc.sync.dma_start(bq[:], bq_d[:])
            if with_out_bias:
                bout = pp.tile([1, D], BF16, name="bout", tag="bout")
                nc.sync.dma_start(bout[:], bout_d[:])

            # collective bounce buffers: two token-half chunks of [ek | ekv]
            cc_in = [dram.tile([TOK, D], BF16, name=f"cc_in{x}") for x in range(2)]
            cc_out = [dram.tile([N_CORES * TOK, D], BF16, name=f"cc_out{x}",
                                addr_space="Shared") for x in range(2)]

            # ---- phase A: qkv projection, k/v first ----
            with tc.tile_pool(name="phaseA", bufs=1) as pa:
                dataT = [pa.tile([128, TOK], BF16, name=f"dataT{k}",
                                 tag=f"dataT{k}") for k in range(KT)]
                wkv = [pa.tile([128, 2 * D], BF16, name=f"wkv{k}",
                               tag=f"wkv{k}") for k in range(KT)]
                wq = [pa.tile([128, D], BF16, name=f"wq{k}", tag=f"wq{k}")
                      for k in range(KT)]
                # kv weights first so pass 1 can start after ~6MB of DMA
                for k in range(KT):
                    nc.sync.dma_start(dataT[k][:], dataT_d[k * 128:(k + 1) * 128, :])
                    nc.sync.dma_start(wkv[k][:], wkv_d[k * 128:(k + 1) * 128, :])
                for k in range(KT):
                    nc.sync.dma_start(wq[k][:], wq_d[k * 128:(k + 1) * 128, :])

                # pass 1: k and v chunks -> exp(k), exp(k)*v -> cc_in -> AG
                for m in range(NG):  # token tile
                    ek = pa.tile([128, D], BF16, name=f"ek{m}", tag="ek", bufs=3)
                    vv = pa.tile([128, D], BF16, name=f"vv{m}", tag="vv", bufs=3)
                    ekv = pa.tile([128, D], BF16, name=f"ekv{m}", tag="ekv", bufs=3)
                    ps = [psp.tile([128, 512], F32, name=f"ps{m}_{i}",
                                   tag="ps") for i in range(4)]
                    for k in range(KT):
                        for i in range(4):
                            nc.tensor.matmul(
                                ps[i][:], dataT[k][:, m * 128:(m + 1) * 128],
                                wkv[k][:, i * 512:(i + 1) * 512],
                                start=(k == 0),
                                stop=(k == KT - 1 and not with_qkv_bias),
                            )
                    if with_qkv_bias:
                        for i in range(4):
                            nc.tensor.matmul(
                                ps[i][:], ones1[:], bkv[:, i * 512:(i + 1) * 512],
                                start=False, stop=True,
                            )
                    for i in range(2):
                        nc.scalar.activation(
                            ek[:, i * 512:(i + 1) * 512], ps[i][:], AF.Exp)
                        nc.vector.tensor_copy(
                            vv[:, i * 512:(i + 1) * 512], ps[2 + i][:])
                    nc.vector.tensor_mul(ekv[:], ek[:], vv[:])
                    # chunk x = m//4 holds token rows [x*512,(x+1)*512):
                    # layout [ek half | ekv half]
                    x, mm = m // 4, m % 4
                    nc.sync.dma_start(
                        cc_in[x][mm * 128:(mm + 1) * 128, :], ek[:])
                    nc.sync.dma_start(
                        cc_in[x][512 + mm * 128:512 + (mm + 1) * 128, :], ekv[:])
                    if m in (3, 7):
                        nc.gpsimd.collective_compute(
                            "AllGather", mybir.AluOpType.bypass,
                            replica_groups=[list(range(N_CORES))],
                            ins=[cc_in[m // 4][:].opt()],
                            outs=[cc_out[m // 4][:].opt()],
                        )

                # exp(pbT) — loads ride behind the cc_in stores, done ~mid-AG
                for t in range(T // 128):
                    praw = pa.tile([128, TOK], BF16, name=f"praw{t}", tag="praw",
                                   bufs=4)
                    nc.sync.dma_start(praw[:], pbT_d[t * 128:(t + 1) * 128, :])
                    nc.scalar.activation(pbe[t][:], praw[:], AF.Exp)

                # pass 2: q chunks + sigmoid — overlaps the collectives;
                # the second half is emitted after the even pass to fill the
                # wait-for-AG1 bubble
                def q_group(m):
                    sq = sq_t[m]
                    ps = [psp.tile([128, 512], F32, name=f"psq{m}_{i}",
                                   tag="ps") for i in range(2)]
                    for k in range(KT):
                        for i in range(2):
                            nc.tensor.matmul(
                                ps[i][:], dataT[k][:, m * 128:(m + 1) * 128],
                                wq[k][:, i * 512:(i + 1) * 512],
                                start=(k == 0),
                                stop=(k == KT - 1 and not with_qkv_bias),
                            )
                    if with_qkv_bias:
                        for i in range(2):
                            nc.tensor.matmul(
                                ps[i][:], ones1[:], bq[:, i * 512:(i + 1) * 512],
                                start=False, stop=True,
                            )
                    for i in range(2):
                        nc.scalar.activation(
                            sq[:, i * 512:(i + 1) * 512], ps[i][:], AF.Sigmoid)

                for m in range(NG):
                    q_group(m)

                # wout/gidx: needed only by phase B, much later
                nc.sync.dma_start(gidx[:], gidx_d[:])
                for k in range(KT):
                    nc.sync.dma_start(wout[k][:], wout_d[k * 128:(k + 1) * 128, :])

            # ---- phase B: num/den + y + output projection ----
            # pbe tiles are SLOT-ordered (host-permuted pbT rows):
            #   s in [0,8):   chunk-0 j-tile s (absolute half s//4, tile s%4)
            #   s in [8,12):  chunk-1 OWN-half tile s-8   (local in cc_in[1])
            #   s in [12,16): chunk-1 OTHER-half tile s-12 (needs AG chunk 1)
            # gidx cols: 0-15 chunk-0 (half*8+u), 16-23 chunk-1-other
            # (half*4+o). The own-half chunk-1 tiles are read straight from
            # this core's cc_in[1], before the second collective lands.
            with tc.tile_pool(name="phaseB", bufs=1) as pbp:
                def gload(x, cols, nt, tagp):
                    tiles = []
                    for u in range(nt):
                        g = pbp.tile([128, 2048], BF16, name=f"ekg{tagp}{u}",
                                     tag="ekg", bufs=12)
                        for half in range(2):
                            col = cols + half * nt + u
                            nc.gpsimd.indirect_dma_start(
                                out=g[:, half * D:(half + 1) * D],
                                out_offset=None,
                                in_=cc_out[x][:],
                                in_offset=bass.IndirectOffsetOnAxis(
                                    ap=gidx[:, col:col + 1], axis=0),
                            )
                        tiles.append(g)
                    return tiles

                spill = {}

                def nd_mms(m2, tiles, merge_sp, tagp):
                    pn = [psp.tile([128, 512], F32, name=f"pn{tagp}{m2}{i}",
                                   tag="ps") for i in range(2)]
                    pd = [psp.tile([128, 512], F32, name=f"pd{tagp}{m2}{i}",
                                   tag="ps") for i in range(2)]
                    nt = len(tiles)
                    for u in range(nt):
                        tile_u, slot = tiles[u]
                        pb_t = pbe[slot]
                        for i in range(2):
                            nc.tensor.matmul(
                                pn[i][:], pb_t[:, m2 * 128:(m2 + 1) * 128],
                                tile_u[:, D + i * 512:D + (i + 1) * 512],
                                start=(u == 0),
                                stop=(u == nt - 1 and merge_sp is None))
                            nc.tensor.matmul(
                                pd[i][:], pb_t[:, m2 * 128:(m2 + 1) * 128],
                                tile_u[:, i * 512:(i + 1) * 512],
                                start=(u == 0),
                                stop=(u == nt - 1 and merge_sp is None))
                    if merge_sp is not None:
                        for i in range(2):
                            nc.tensor.matmul(
                                pn[i][:], ident[:],
                                merge_sp[:, i * 512:(i + 1) * 512],
                                start=False, stop=True)
                            nc.tensor.matmul(
                                pd[i][:], ident[:],
                                merge_sp[:, D + i * 512:D + (i + 1) * 512],
                                start=False, stop=True)
                    return pn, pd

                def to_spill(m2, pn, pd):
                    if m2 in spill:
                        sp = spill[m2]
                    else:
                        sp = pbp.tile([128, 4 * 512], BF16, name=f"sp{m2}",
                                      tag=f"sp{m2}")
                        spill[m2] = sp
                    for i in range(2):
                        nc.scalar.copy(sp[:, i * 512:(i + 1) * 512], pn[i][:])
                        nc.scalar.copy(
                            sp[:, D + i * 512:D + (i + 1) * 512], pd[i][:])

                # pass B1: BOTH chunks' own-half tiles, straight from this
                # core's cc_in — runs while the collectives are on the wire
                own = []
                for x in range(2):
                    for o in range(4):
                        g = pbp.tile([128, 2048], BF16, name=f"ekgo{x}{o}",
                                     tag="ekg", bufs=12)
                        nc.sync.dma_start(
                            g[:, :D], cc_in[x][o * 128:(o + 1) * 128, :])
                        nc.sync.dma_start(
                            g[:, D:],
                            cc_in[x][512 + o * 128:512 + (o + 1) * 128, :])
                        own.append((g, 8 * x + o))
                for pair in range(NG // 2):
                    g0, g1 = 2 * pair, 2 * pair + 1
                    a0 = nd_mms(g0, own, None, "A")
                    to_spill(g0, *a0)
                    a1 = nd_mms(g1, own, None, "A")
                    to_spill(g1, *a1)

                # pass B2: chunk-0 other-half tiles (after AG chunk 0),
                # merging the B1 partials back in on the PE
                ekg0 = [(g, 4 + o) for o, g in enumerate(gload(0, 0, 4, "z"))]
                for pair in range(NG // 2):
                    g0, g1 = 2 * pair, 2 * pair + 1
                    a0 = nd_mms(g0, ekg0, spill[g0], "B")
                    to_spill(g0, *a0)
                    a1 = nd_mms(g1, ekg0, spill[g1], "B")
                    to_spill(g1, *a1)

                # pass B3: chunk-1 other-half tiles (after AG chunk 1),
                # epilogue, transpose, output projection
                ekg1 = [(g, 12 + o) for o, g in enumerate(gload(1, 8, 4, "o"))]

                def tail(m2, yv):
                    yT = [pbp.tile([128, 128], BF16, name=f"yT{m2}_{k}",
                                   tag=f"yT{k}", bufs=2) for k in range(KT)]
                    for k in range(KT):
                        pt = pstr.tile([128, 128], BF16, name=f"pt{m2}{k}",
                                       tag="tr")
                        nc.tensor.transpose(
                            pt[:], yv[:, k * 128:(k + 1) * 128], ident[:])
                        nc.vector.tensor_copy(yT[k][:], pt[:])
                    po = [psp.tile([128, 512], F32, name=f"po{m2}_{n}", tag="ps")
                          for n in range(2)]
                    for k in range(KT):
                        for n in range(2):
                            nc.tensor.matmul(
                                po[n][:], yT[k][:],
                                wout[k][:, n * 512:(n + 1) * 512],
                                start=(k == 0),
                                stop=(k == KT - 1 and not with_out_bias))
                    if with_out_bias:
                        for n in range(2):
                            nc.tensor.matmul(
                                po[n][:], ones1[:], bout[:, n * 512:(n + 1) * 512],
                                start=False, stop=True)
                    for n in range(2):
                        osb = pbp.tile([128, 512], F32, name=f"osb{m2}_{n}",
                                       tag="osb", bufs=4)
                        nc.scalar.copy(osb[:], po[n][:])
                        nc.sync.dma_start(
                            out_d[m2 * 128:(m2 + 1) * 128,
                                  n * 512:(n + 1) * 512], osb[:])

                def odd_group(m2):
                    pn, pd = nd_mms(m2, ekg1, spill[m2], "C")
                    y = pbp.tile([128, D], BF16, name=f"y{m2}", tag="y", bufs=3)
                    for i in range(2):
                        tn = pbp.tile([128, 512], F32, name=f"tn{m2}{i}",
                                      tag="tn", bufs=3)
                        rec = pbp.tile([128, 512], F32, name=f"rc{m2}{i}",
                                       tag="rc", bufs=3)
                        nc.vector.reciprocal_approx_fast(rec[:], pd[i][:])
                        nc.vector.tensor_mul(tn[:], pn[i][:], rec[:])
                        nc.vector.tensor_mul(
                            y[:, i * 512:(i + 1) * 512], tn[:],
                            sq_t[m2][:, i * 512:(i + 1) * 512])
                    return y

                for pair in range(NG // 2):
                    g0, g1 = 2 * pair, 2 * pair + 1
                    y0 = odd_group(g0)
                    y1 = odd_group(g1)
                    tail(g0, y0)
                    tail(g1, y1)

    nc.compile()
    return nc


def _prep_inputs(data, W_qkv, b_qkv, pos_bias_param, W_out, b_out):
    bf = ml_dtypes.bfloat16
    data = np.asarray(data, np.float32)
    W_qkv = np.asarray(W_qkv, np.float32)
    b_qkv = np.asarray(b_qkv, np.float32)
    pos_bias_param = np.asarray(pos_bias_param, np.float32)
    W_out = np.asarray(W_out, np.float32)
    b_out = np.asarray(b_out, np.float32)

    with_qkv_bias = bool(np.any(b_qkv))
    with_out_bias = bool(np.any(b_out))

    wq = np.ascontiguousarray(W_qkv[:, :D]).astype(bf)
    wkv = np.ascontiguousarray(W_qkv[:, D:]).astype(bf)
    wout = W_out.astype(bf)
    pbT = np.ascontiguousarray(pos_bias_param.T)  # [j, i]

    p = np.arange(128)
    in_maps = []
    for r in range(N_CORES):
        b, h = r // 2, r % 2
        isl = slice(h * TOK, (h + 1) * TOK)
        dT = np.ascontiguousarray(
            data[isl, b, :].T).astype(bf)                    # [d_in, tok]
        pbT_c = np.ascontiguousarray(pbT[:, isl]).astype(bf)  # [j, i_loc]
        # slot-permute pbT rows: per chunk x, slots 0-3 = own half,
        # 4-7 = other half; pbe slot s = 8*x + within-chunk slot
        rows = []
        for s in range(16):
            x, ss = s // 8, s % 8
            hp = h if ss < 4 else 1 - h
            j0 = hp * 1024 + 512 * x + (ss % 4) * 128
            rows.append(pbT_c[j0:j0 + 128])
        pbT_c = np.ascontiguousarray(np.concatenate(rows, axis=0))
        # gidx: cols x*8 + half*4 + o -> chunk-x other-half tile o
        gidx = np.zeros((128, 24), np.int32)
        for x in range(2):
            for half in range(2):
                for o in range(4):
                    base = (2 * b + (1 - h)) * 1024 + half * 512 + o * 128
                    gidx[:, x * 8 + half * 4 + o] = base + p
        m = {"dataT": dT, "wq": wq, "wkv": wkv, "pbT": pbT_c, "wout": wout,
             "gidx": gidx}
        if with_qkv_bias:
            m["bq"] = np.ascontiguousarray(b_qkv[:D]).reshape(1, D).astype(bf)
            m["bkv"] = np.ascontiguousarray(b_qkv[D:]).reshape(1, 2 * D).astype(bf)
        if with_out_bias:
            m["bout"] = b_out.reshape(1, D).astype(bf)
        in_maps.append(m)
    return in_maps, with_qkv_bias, with_out_bias


def run(data, W_qkv, b_qkv, pos_bias_param, W_out, b_out, **spmd_kwargs):
    in_maps, wb, ob = _prep_inputs(data, W_qkv, b_qkv, pos_bias_param, W_out, b_out)
    key = (wb, ob)
    if key not in _cache:
        _cache[key] = build(wb, ob)
    nc = _cache[key]
    res = run_bass_kernel_spmd(nc, in_maps, core_ids=list(range(N_CORES)),
                               **spmd_kwargs)
    out = np.empty((T, B, D), np.float32)
    for r in range(N_CORES):
        b, h = r // 2, r % 2
        out[h * TOK:(h + 1) * TOK, b, :] = res.results[r]["out"]
    return out, res


def kernel(data, W_qkv, b_qkv, pos_bias_param, W_out, b_out):
    out, _ = run(data, W_qkv, b_qkv, pos_bias_param, W_out, b_out)
    return out
